# revision 23
# baseline (speedup 1.0000x reference)
"""Per-donor routed linear layer on 8 Trainium2 cores — int8-quantized x.

out[i] = x[i] @ W[donor_labels[i]].T + b[donor_labels[i]]

x is quantized per-row to int8 on the host (s_i = absmax(x_i)/127,
q = rint(x/s)), halving HBM read traffic (the kernel is memory-bound).
The device computes raw[j,i] = sum_k W16[j,k] * q[i,k] with W in fp16;
the host applies out = s_i * t_j * raw + b afterwards.

v5 structure:
- x HBM layout is partition-major and unit-slab ordered: each unit
  (pair of 512-row blocks, or a single block at the pipeline head/tail)
  is a contiguous per-partition slab in k-major order [k][b][r].  A
  multi-unit chunk is one contiguous DMA.
- Per pair, ONE DVE cast op covers the first DVE_COLS 512-element
  columns of the slab and ONE ACT op the rest (column split balances the
  two engines finer than k-tile granularity); every matmul k-tile has
  exactly one producer.
- PSUM pair tiles (2 banks); one paired ACT eviction per pair with a
  per-gene int8 output scale. int8 output halves store traffic.
"""

import os
import sys

sys.path.insert(0, "/opt/trn_rl_repo")

import numpy as np

import concourse.bacc as bacc
import concourse.mybir as mybir
from concourse.tile import TileContext
from concourse.bass_utils import run_bass_kernel_spmd

N_CORES = 8
N_DONORS = 8
D_IN = 1024
N_GENES = 100
K_TILES = D_IN // 128
BLOCK = 512
BB = K_TILES * BLOCK  # int8 bytes per block per partition

DVE_COLS = int(os.environ.get("V5_DVE_COLS", "12"))  # of 16 pair columns on DVE
DVE_COLS1 = int(os.environ.get("V5_DVE_COLS1", "5"))  # of 8 single columns on DVE
CHUNK = int(os.environ.get("V5_CHUNK", "2"))  # blocks per x DMA
OG = int(os.environ.get("V5_OG", "4"))  # output blocks per store DMA
LAG = int(os.environ.get("V5_LAG", "1"))  # units evictions trail compute by
OUT_DT = os.environ.get("V5_OUT_DT", "int8")  # int8 | fp16
OUT_ISSUE = os.environ.get("V5_OUT_ISSUE", "sync")  # out-store DMA issuer
X16_BUFS = int(os.environ.get("V5_X16_BUFS", "4"))
X8_BUFS = int(os.environ.get("V5_X8_BUFS", "4"))
PSUM_BUFS = int(os.environ.get("V5_PSUM_BUFS", "3"))
OUT_C = float(os.environ.get("V5_OUT_C", "5.6"))  # sigmas of int8-out range


def _chunks_and_units(n_blocks: int):
    """chunks: list of (j0, w) DMA extents; units: list of (j0, w) with w in
    {1, 2}; units never straddle chunk boundaries; the final chunk/unit is
    always a single block (short drain, partial-width tail)."""
    lead = [1, 1, 2]
    chunks = []
    left = n_blocks
    j = 0
    for g in lead:
        if left <= 1:
            break
        g = min(g, left - 1)
        chunks.append((j, g))
        j += g
        left -= g
    while left > 1:
        g = min(CHUNK, left - 1)
        if g == 2 and left - g == 1:
            pass  # keep pairs; remainder single handled below
        chunks.append((j, g))
        j += g
        left -= g
    if left == 1:
        chunks.append((j, 1))
    units = []
    for j0, g in chunks:
        b = 0
        while b < g:
            w = min(2, g - b)
            units.append((j0 + b, w))
            b += w
    assert units[-1][1] == 1
    return chunks, units


def _build_program(R: int):
    nc = bacc.Bacc(
        "TRN2",
        target_bir_lowering=False,
        debug=False,
        enable_asserts=False,
        num_devices=N_CORES,
    )
    n_blocks = -(-R // BLOCK)
    bw_last = R - (n_blocks - 1) * BLOCK  # width of the final (partial) block
    chunks, units = _chunks_and_units(n_blocks)

    def blk_w(j):
        return bw_last if j == n_blocks - 1 else BLOCK

    def col_off(j):  # int8 bytes per partition up to block j
        return j * BB

    out_dt = mybir.dt.int8 if OUT_DT == "int8" else mybir.dt.float16
    xin_cols = col_off(n_blocks - 1) + K_TILES * bw_last

    xin = nc.dram_tensor(
        "xin", (128, xin_cols), mybir.dt.int8, kind="ExternalInput"
    ).ap()
    wb = nc.dram_tensor(
        "wb", (128, K_TILES, N_GENES), mybir.dt.float16, kind="ExternalInput"
    ).ap()
    osc = nc.dram_tensor(
        "osc", (N_GENES, 1), mybir.dt.float32, kind="ExternalInput"
    ).ap()
    outt = nc.dram_tensor("outt", (N_GENES, R), out_dt, kind="ExternalOutput").ap()

    # out-store group sizes: OG blocks each, tiny final group for short drain
    sizes_out = []
    left = n_blocks
    while left > 0:
        g = min(OG, left)
        sizes_out.append(g)
        left -= g
    if sizes_out[-1] > 1:
        sizes_out[-1] -= 1
        sizes_out.append(1)
    out_group_of = {}
    j0 = 0
    for g in sizes_out:
        out_group_of[j0] = g
        j0 += g

    with TileContext(nc) as tc:
        with (
            tc.tile_pool(name="const", bufs=1) as const_pool,
            tc.tile_pool(name="x8p", bufs=X8_BUFS) as x8_pool,
            tc.tile_pool(name="x16p", bufs=X16_BUFS) as x16_pool,
            tc.tile_pool(name="op", bufs=3) as out_pool,
            tc.tile_pool(name="ps", bufs=PSUM_BUFS, space="PSUM") as psum_pool,
        ):
            w16 = const_pool.tile([128, K_TILES, N_GENES], mybir.dt.float16)
            nc.scalar.dma_start(out=w16[:], in_=wb)
            oscale = const_pool.tile([N_GENES, 1], mybir.dt.float32)
            nc.scalar.dma_start(out=oscale[:], in_=osc)
            # tiny warmup ops: absorb DVE/ACT microcode library loads while
            # the first x DMA is in flight
            warm = const_pool.tile([128, 8], mybir.dt.float16, tag="warm")
            nc.vector.memset(warm[:], 0.0)
            nc.scalar.copy(out=warm[:, :4], in_=warm[:, 4:])

            evict_state = {"o_tile": None, "g0j": None, "gsize": None}

            def emit_evict(j, w, psum):
                if j in out_group_of:
                    evict_state["gsize"] = out_group_of[j]
                    evict_state["g0j"] = j
                    evict_state["o_tile"] = out_pool.tile(
                        [N_GENES, OG, BLOCK], out_dt, name="o", tag="o"
                    )
                o_tile = evict_state["o_tile"]
                g0j = evict_state["g0j"]
                gsize = evict_state["gsize"]
                lo = j - g0j
                bw = blk_w(j + w - 1)  # only the final unit can be partial
                if w == 1:
                    esrc, edst = psum[:, 0, :bw], o_tile[:, lo, :bw]
                else:
                    esrc, edst = psum[:, :w, :], o_tile[:, lo : lo + w, :]
                if OUT_DT == "int8":
                    nc.scalar.activation(
                        out=edst,
                        in_=esrc,
                        func=mybir.ActivationFunctionType.Copy,
                        scale=oscale[:],
                    )
                else:
                    nc.scalar.copy(out=edst, in_=esrc)
                if lo + w == gsize:
                    g0 = g0j * BLOCK
                    gw = min(gsize * BLOCK, R - g0)
                    src = o_tile.rearrange("p g r -> p (g r)")[:, :gw]
                    if OUT_ISSUE == "scalar":
                        nc.scalar.dma_start(out=outt[:, g0 : g0 + gw], in_=src)
                    else:
                        nc.sync.dma_start(out=outt[:, g0 : g0 + gw], in_=src)

            # x8 chunk tiles are indexed by flat column (one col = blk-width int8)
            ui = 0
            pending = []  # (j, w, psum) awaiting eviction
            for ci, (cj0, cg) in enumerate(chunks):
                cbw = blk_w(cj0 + cg - 1)  # partial only on the final single chunk
                x8c = x8_pool.tile(
                    [128, cg * K_TILES, cbw], mybir.dt.int8, name="x8", tag="x8"
                )
                c0 = col_off(cj0)
                if ci == 0:
                    # fine-grained first chunk: the first matmuls chase the
                    # smallest possible DMA+cast chain
                    for a, b2 in ((0, 3), (3, 6), (6, K_TILES)):
                        nc.sync.dma_start(
                            out=x8c[:, a:b2],
                            in_=xin[:, c0 + a * cbw : c0 + b2 * cbw],
                        )
                else:
                    nc.sync.dma_start(
                        out=x8c.rearrange("p c r -> p (c r)"),
                        in_=xin[:, c0 : c0 + cg * K_TILES * cbw],
                    )
                # units inside this chunk
                off = 0  # column offset within chunk
                while ui < len(units) and units[ui][0] < cj0 + cg:
                    j, w = units[ui]
                    bw = blk_w(j + w - 1)
                    ncols = w * K_TILES
                    dc = DVE_COLS if w == 2 else DVE_COLS1
                    # x16 slab tile, k-major: [128, K_TILES, w, BLOCK]
                    x16 = x16_pool.tile(
                        [128, K_TILES, w, BLOCK], mybir.dt.float16, name="x16", tag="x16"
                    )
                    xsrc = x8c[:, off : off + ncols]  # [128, ncols, cbw]
                    if w == 2:
                        dst = x16.rearrange("p k b r -> p (k b) r")
                    else:
                        dst = x16[:, :, 0, :bw] if bw < BLOCK else x16[
                            :, :, 0
                        ]
                    if ui == 0:
                        # split the first cast so matmul k=0 starts earliest
                        nc.vector.tensor_copy(out=dst[:, :3], in_=xsrc[:, :3])
                        nc.vector.tensor_copy(out=dst[:, 3:dc], in_=xsrc[:, 3:dc])
                    else:
                        nc.vector.tensor_copy(out=dst[:, :dc], in_=xsrc[:, :dc])
                    nc.scalar.copy(out=dst[:, dc:ncols], in_=xsrc[:, dc:ncols])
                    psum = psum_pool.tile(
                        [N_GENES, 2, BLOCK], mybir.dt.float32, name="ps", tag="ps"
                    )
                    for bi in range(w):
                        for k in range(K_TILES):
                            nc.tensor.matmul(
                                out=psum[:, bi, :bw],
                                lhsT=w16[:, k, :],
                                rhs=x16[:, k, bi, :bw],
                                start=(k == 0),
                                stop=(k == K_TILES - 1),
                            )
                    pending.append((j, w, psum))
                    if len(pending) > LAG:
                        emit_evict(*pending.pop(0))
                    off += ncols
                    ui += 1
            for item in pending:
                emit_evict(*item)

    nc.compile()
    return nc


def kernel(x, donor_labels, W, b):
    x = np.ascontiguousarray(x, dtype=np.float32)
    labels = np.asarray(donor_labels).astype(np.int64)
    W = np.asarray(W, dtype=np.float32)
    b = np.asarray(b, dtype=np.float32)
    B = x.shape[0]

    # per-row int8 quantization (host): x ~= s[:,None] * q
    s = np.abs(x).max(axis=1) / 127.0
    np.maximum(s, 1e-30, out=s)
    q_full = np.rint(x / s[:, None]).astype(np.int8)

    order = np.argsort(labels, kind="stable")
    counts = np.bincount(labels, minlength=N_DONORS)
    starts = np.zeros(N_DONORS + 1, dtype=np.int64)
    np.cumsum(counts, out=starts[1:])
    R = max(BLOCK, int(-(-counts.max() // 64)) * 64)
    n_blocks = -(-R // BLOCK)
    R_pad = n_blocks * BLOCK
    _, units = _chunks_and_units(n_blocks)

    # int8 output scale: raw[j,i] = W16[j].q_i has std ~ ||W16[j]|| ||q_i||/32;
    # range t_j covers OUT_C sigmas at the worst row norm.
    W16 = W.astype(np.float16)
    wnorm = np.linalg.norm(W16.astype(np.float32), axis=2)  # (8, 100)
    qmax = np.sqrt(np.max((q_full.astype(np.float32) ** 2).sum(axis=1)))
    t = wnorm * (qmax / 32.0) * OUT_C / 127.0  # (8, 100)
    np.maximum(t, 1e-30, out=t)

    in_maps = []
    idx_per_core = []
    for d in range(N_CORES):
        idx = order[starts[d] : starts[d + 1]]
        idx_per_core.append(idx)
        qr = np.zeros((R_pad, D_IN), dtype=np.int8)
        qr[: len(idx)] = q_full[idx]
        # per-unit k-major slabs: unit (j0,w) -> [128, k, b, r] flattened;
        # the final unit is partial-width (bw rows instead of 512)
        slabs = []
        for j0, w in units:
            bw = min(BLOCK, R - j0 * BLOCK) if w == 1 else BLOCK
            u = qr[j0 * BLOCK : j0 * BLOCK + (w - 1) * BLOCK + bw]
            u = u.reshape(w, bw, K_TILES, 128).transpose(3, 2, 0, 1)  # p,k,b,r
            slabs.append(np.ascontiguousarray(u).reshape(128, w * K_TILES * bw))
        qb = np.ascontiguousarray(np.concatenate(slabs, axis=1))
        in_maps.append(
            {
                "xin": qb,
                "wb": np.ascontiguousarray(
                    W[d].T.reshape(K_TILES, 128, N_GENES).transpose(1, 0, 2)
                ).astype(np.float16),
                "osc": np.ascontiguousarray(
                    (1.0 / t[d]).reshape(N_GENES, 1)
                ).astype(np.float32),
            }
        )

    nc = _build_program(R)

    try:
        res = run_bass_kernel_spmd(nc, in_maps, core_ids=list(range(N_CORES)))
    except Exception:
        # One retry: the axon-tunneled device occasionally drops a run.
        res = run_bass_kernel_spmd(nc, in_maps, core_ids=list(range(N_CORES)))

    out = np.empty((B, N_GENES), dtype=np.float32)
    for d in range(N_CORES):
        idx = idx_per_core[d]
        raw = res.results[d]["outt"][:, : len(idx)].T.astype(np.float32)
        if OUT_DT == "int8":
            raw *= t[d][None, :]
        out[idx] = raw * s[idx][:, None] + b[d][None, :]
    return out


# revision 30
# speedup vs baseline: 1.1713x; 1.1713x over previous
"""Per-donor routed linear layer on 8 Trainium2 cores — int8-quantized x.

out[i] = x[i] @ W[donor_labels[i]].T + b[donor_labels[i]]

x is quantized per-row to int8 on the host (s_i = absmax(x_i)/127,
q = rint(x/s)), halving HBM read traffic (the kernel is memory-bound).
The device computes raw[j,i] = sum_k W16[j,k] * q[i,k] with W in fp16;
the host applies out = s_i * t_j * raw + b afterwards.

v5 structure:
- x HBM layout is partition-major and unit-slab ordered: each unit
  (pair of 512-row blocks, or a single block at the pipeline head/tail)
  is a contiguous per-partition slab in k-major order [k][b][r].  A
  multi-unit chunk is one contiguous DMA.
- Per pair, ONE DVE cast op covers the first DVE_COLS 512-element
  columns of the slab and ONE ACT op the rest (column split balances the
  two engines finer than k-tile granularity); every matmul k-tile has
  exactly one producer.
- PSUM pair tiles (2 banks); one paired ACT eviction per pair with a
  per-gene int8 output scale. int8 output halves store traffic.
"""

import os
import sys

sys.path.insert(0, "/opt/trn_rl_repo")

import numpy as np

import concourse.bacc as bacc
import concourse.mybir as mybir
from concourse.tile import TileContext
from concourse.bass_utils import run_bass_kernel_spmd

N_CORES = 8
N_DONORS = 8
D_IN = 1024
N_GENES = 100
K_TILES = D_IN // 128
BLOCK = 512
BB = K_TILES * BLOCK  # int8 bytes per block per partition

DVE_COLS = int(os.environ.get("V5_DVE_COLS", "12"))  # of 16 pair columns on DVE
DVE_COLS1 = int(os.environ.get("V5_DVE_COLS1", "6"))  # of 8 single columns on DVE
CHUNK = int(os.environ.get("V5_CHUNK", "2"))  # blocks per x DMA
OG = int(os.environ.get("V5_OG", "4"))  # output blocks per store DMA
LAG = int(os.environ.get("V5_LAG", "1"))  # units evictions trail compute by
OUT_DT = os.environ.get("V5_OUT_DT", "int8")  # int8 | fp16
OUT_ISSUE = os.environ.get("V5_OUT_ISSUE", "sync")  # out-store DMA issuer
X16_BUFS = int(os.environ.get("V5_X16_BUFS", "4"))
X8_BUFS = int(os.environ.get("V5_X8_BUFS", "4"))
PSUM_BUFS = int(os.environ.get("V5_PSUM_BUFS", "3"))
OUT_C = float(os.environ.get("V5_OUT_C", "5.6"))  # sigmas of int8-out range


def _chunks_and_units(n_blocks: int):
    """chunks: list of (j0, w) DMA extents; units: list of (j0, w) with w in
    {1, 2}; units never straddle chunk boundaries; the final chunk/unit is
    always a single block (short drain, partial-width tail)."""
    lead = [1, 1, 2]
    chunks = []
    left = n_blocks
    j = 0
    for g in lead:
        if left <= 1:
            break
        g = min(g, left - 1)
        chunks.append((j, g))
        j += g
        left -= g
    while left > 1:
        g = min(CHUNK, left - 1)
        if g == 2 and left - g == 1:
            pass  # keep pairs; remainder single handled below
        chunks.append((j, g))
        j += g
        left -= g
    if left == 1:
        chunks.append((j, 1))
    units = []
    for j0, g in chunks:
        b = 0
        while b < g:
            w = min(2, g - b)
            units.append((j0 + b, w))
            b += w
    assert units[-1][1] == 1
    return chunks, units


def _build_program(R: int):
    nc = bacc.Bacc(
        "TRN2",
        target_bir_lowering=False,
        debug=False,
        enable_asserts=False,
        num_devices=N_CORES,
    )
    n_blocks = -(-R // BLOCK)
    bw_last = R - (n_blocks - 1) * BLOCK  # width of the final (partial) block
    chunks, units = _chunks_and_units(n_blocks)

    def blk_w(j):
        return bw_last if j == n_blocks - 1 else BLOCK

    def col_off(j):  # int8 bytes per partition up to block j
        return j * BB

    out_dt = mybir.dt.int8 if OUT_DT == "int8" else mybir.dt.float16
    xin_cols = col_off(n_blocks - 1) + K_TILES * bw_last

    xin = nc.dram_tensor(
        "xin", (128, xin_cols), mybir.dt.int8, kind="ExternalInput"
    ).ap()
    wb = nc.dram_tensor(
        "wb", (128, K_TILES, N_GENES), mybir.dt.float16, kind="ExternalInput"
    ).ap()
    osc = nc.dram_tensor(
        "osc", (N_GENES, 1), mybir.dt.float32, kind="ExternalInput"
    ).ap()
    outt = nc.dram_tensor("outt", (N_GENES, R), out_dt, kind="ExternalOutput").ap()

    # out-store group sizes: OG blocks each, tiny final group for short drain
    sizes_out = []
    left = n_blocks
    while left > 0:
        g = min(OG, left)
        sizes_out.append(g)
        left -= g
    if sizes_out[-1] > 1:
        sizes_out[-1] -= 1
        sizes_out.append(1)
    out_group_of = {}
    j0 = 0
    for g in sizes_out:
        out_group_of[j0] = g
        j0 += g

    with TileContext(nc) as tc:
        with (
            tc.tile_pool(name="const", bufs=1) as const_pool,
            tc.tile_pool(name="x8p", bufs=X8_BUFS) as x8_pool,
            tc.tile_pool(name="x16p", bufs=X16_BUFS) as x16_pool,
            tc.tile_pool(name="op", bufs=3) as out_pool,
            tc.tile_pool(name="ps", bufs=PSUM_BUFS, space="PSUM") as psum_pool,
        ):
            w16 = const_pool.tile([128, K_TILES, N_GENES], mybir.dt.float16)
            nc.scalar.dma_start(out=w16[:], in_=wb)
            oscale = const_pool.tile([N_GENES, 1], mybir.dt.float32)
            nc.scalar.dma_start(out=oscale[:], in_=osc)
            # tiny warmup ops: absorb DVE/ACT microcode library loads while
            # the first x DMA is in flight
            warm = const_pool.tile([128, 8], mybir.dt.float16, tag="warm")
            nc.vector.memset(warm[:], 0.0)
            nc.scalar.copy(out=warm[:, :4], in_=warm[:, 4:])

            evict_state = {"o_tile": None, "g0j": None, "gsize": None}

            def emit_evict(j, w, psum):
                if j in out_group_of:
                    evict_state["gsize"] = out_group_of[j]
                    evict_state["g0j"] = j
                    evict_state["o_tile"] = out_pool.tile(
                        [N_GENES, OG, BLOCK], out_dt, name="o", tag="o"
                    )
                o_tile = evict_state["o_tile"]
                g0j = evict_state["g0j"]
                gsize = evict_state["gsize"]
                lo = j - g0j
                bw = blk_w(j + w - 1)  # only the final unit can be partial
                if w == 1:
                    esrc, edst = psum[:, 0, :bw], o_tile[:, lo, :bw]
                else:
                    esrc, edst = psum[:, :w, :], o_tile[:, lo : lo + w, :]
                if OUT_DT == "int8":
                    nc.scalar.activation(
                        out=edst,
                        in_=esrc,
                        func=mybir.ActivationFunctionType.Copy,
                        scale=oscale[:],
                    )
                else:
                    nc.scalar.copy(out=edst, in_=esrc)
                if lo + w == gsize:
                    g0 = g0j * BLOCK
                    gw = min(gsize * BLOCK, R - g0)
                    src = o_tile.rearrange("p g r -> p (g r)")[:, :gw]
                    if OUT_ISSUE == "scalar":
                        nc.scalar.dma_start(out=outt[:, g0 : g0 + gw], in_=src)
                    else:
                        nc.sync.dma_start(out=outt[:, g0 : g0 + gw], in_=src)

            # x8 chunk tiles are indexed by flat column (one col = blk-width int8)
            ui = 0
            pending = []  # (j, w, psum) awaiting eviction
            for ci, (cj0, cg) in enumerate(chunks):
                cbw = blk_w(cj0 + cg - 1)  # partial only on the final single chunk
                x8c = x8_pool.tile(
                    [128, cg * K_TILES, cbw], mybir.dt.int8, name="x8", tag="x8"
                )
                c0 = col_off(cj0)
                if ci == 0:
                    # fine-grained first chunk: the first matmuls chase the
                    # smallest possible DMA+cast chain
                    for a, b2 in ((0, 3), (3, 6), (6, K_TILES)):
                        nc.sync.dma_start(
                            out=x8c[:, a:b2],
                            in_=xin[:, c0 + a * cbw : c0 + b2 * cbw],
                        )
                else:
                    nc.sync.dma_start(
                        out=x8c.rearrange("p c r -> p (c r)"),
                        in_=xin[:, c0 : c0 + cg * K_TILES * cbw],
                    )
                # units inside this chunk
                off = 0  # column offset within chunk
                while ui < len(units) and units[ui][0] < cj0 + cg:
                    j, w = units[ui]
                    bw = blk_w(j + w - 1)
                    ncols = w * K_TILES
                    dc = DVE_COLS if w == 2 else DVE_COLS1
                    # x16 slab tile, k-major: [128, K_TILES, 2, BLOCK]; plain
                    # tile slices only (k-granular DVE/ACT split for pairs)
                    x16 = x16_pool.tile(
                        [128, K_TILES, 2, BLOCK], mybir.dt.float16, name="x16", tag="x16"
                    )
                    xsrc = x8c[:, off : off + ncols]  # [128, ncols, cbw]
                    if w == 2:
                        kd = dc // 2  # k-tiles on DVE
                        nc.vector.tensor_copy(
                            out=x16[:, :kd], in_=xsrc[:, : 2 * kd]
                        )
                        nc.scalar.copy(out=x16[:, kd:], in_=xsrc[:, 2 * kd :])
                    else:
                        dst = x16[:, :, 0, :bw]
                        if ui == 0:
                            # split the first cast: matmul k=0 starts earliest
                            nc.vector.tensor_copy(out=dst[:, :3], in_=xsrc[:, :3])
                            nc.vector.tensor_copy(
                                out=dst[:, 3:dc], in_=xsrc[:, 3:dc]
                            )
                        else:
                            nc.vector.tensor_copy(out=dst[:, :dc], in_=xsrc[:, :dc])
                        nc.scalar.copy(out=dst[:, dc:ncols], in_=xsrc[:, dc:ncols])
                    psum = psum_pool.tile(
                        [N_GENES, 2, BLOCK], mybir.dt.float32, name="ps", tag="ps"
                    )
                    for bi in range(w):
                        for k in range(K_TILES):
                            nc.tensor.matmul(
                                out=psum[:, bi, :bw],
                                lhsT=w16[:, k, :],
                                rhs=x16[:, k, bi if w == 2 else 0, :bw],
                                start=(k == 0),
                                stop=(k == K_TILES - 1),
                            )
                    pending.append((j, w, psum))
                    if len(pending) > LAG:
                        emit_evict(*pending.pop(0))
                    off += ncols
                    ui += 1
            for item in pending:
                emit_evict(*item)

    nc.compile()
    return nc


def kernel(x, donor_labels, W, b):
    x = np.ascontiguousarray(x, dtype=np.float32)
    labels = np.asarray(donor_labels).astype(np.int64)
    W = np.asarray(W, dtype=np.float32)
    b = np.asarray(b, dtype=np.float32)
    B = x.shape[0]

    # per-row int8 quantization (host): x ~= s[:,None] * q
    s = np.abs(x).max(axis=1) / 127.0
    np.maximum(s, 1e-30, out=s)
    q_full = np.rint(x / s[:, None]).astype(np.int8)

    order = np.argsort(labels, kind="stable")
    counts = np.bincount(labels, minlength=N_DONORS)
    starts = np.zeros(N_DONORS + 1, dtype=np.int64)
    np.cumsum(counts, out=starts[1:])
    R = max(BLOCK, int(-(-counts.max() // 64)) * 64)
    n_blocks = -(-R // BLOCK)
    R_pad = n_blocks * BLOCK
    _, units = _chunks_and_units(n_blocks)

    # int8 output scale: raw[j,i] = W16[j].q_i has std ~ ||W16[j]|| ||q_i||/32;
    # range t_j covers OUT_C sigmas at the worst row norm.
    W16 = W.astype(np.float16)
    wnorm = np.linalg.norm(W16.astype(np.float32), axis=2)  # (8, 100)
    qmax = np.sqrt(np.max((q_full.astype(np.float32) ** 2).sum(axis=1)))
    t = wnorm * (qmax / 32.0) * OUT_C / 127.0  # (8, 100)
    np.maximum(t, 1e-30, out=t)

    in_maps = []
    idx_per_core = []
    for d in range(N_CORES):
        idx = order[starts[d] : starts[d + 1]]
        idx_per_core.append(idx)
        qr = np.zeros((R_pad, D_IN), dtype=np.int8)
        qr[: len(idx)] = q_full[idx]
        # per-unit k-major slabs: unit (j0,w) -> [128, k, b, r] flattened;
        # the final unit is partial-width (bw rows instead of 512)
        slabs = []
        for j0, w in units:
            bw = min(BLOCK, R - j0 * BLOCK) if w == 1 else BLOCK
            u = qr[j0 * BLOCK : j0 * BLOCK + (w - 1) * BLOCK + bw]
            u = u.reshape(w, bw, K_TILES, 128).transpose(3, 2, 0, 1)  # p,k,b,r
            slabs.append(np.ascontiguousarray(u).reshape(128, w * K_TILES * bw))
        qb = np.ascontiguousarray(np.concatenate(slabs, axis=1))
        in_maps.append(
            {
                "xin": qb,
                "wb": np.ascontiguousarray(
                    W[d].T.reshape(K_TILES, 128, N_GENES).transpose(1, 0, 2)
                ).astype(np.float16),
                "osc": np.ascontiguousarray(
                    (1.0 / t[d]).reshape(N_GENES, 1)
                ).astype(np.float32),
            }
        )

    nc = _build_program(R)

    def run_once():
        try:
            res = run_bass_kernel_spmd(nc, in_maps, core_ids=list(range(N_CORES)))
        except Exception:
            # One retry: the axon-tunneled device occasionally drops a run.
            res = run_bass_kernel_spmd(nc, in_maps, core_ids=list(range(N_CORES)))
        out = np.empty((B, N_GENES), dtype=np.float32)
        for d in range(N_CORES):
            idx = idx_per_core[d]
            raw = res.results[d]["outt"][:, : len(idx)].T.astype(np.float32)
            if OUT_DT == "int8":
                raw *= t[d][None, :]
            out[idx] = raw * s[idx][:, None] + b[d][None, :]
        return out

    # sampled host check guards against a rare dropped/corrupted device run
    rng = np.random.default_rng(0)
    rows = rng.choice(B, size=64, replace=False)
    exp = (
        np.einsum("rd,rgd->rg", x[rows], W[labels[rows]]) + b[labels[rows]]
    )
    scale = max(1e-9, float(np.abs(exp).max()))
    for _ in range(2):
        out = run_once()
        if float(np.abs(out[rows] - exp).max()) / scale < 0.05:
            break
    return out
